# revision 35
# baseline (speedup 1.0000x reference)
"""Trainium2 Bass kernel for nn_CrossAttentionFusion — fp8 DoubleRow version.

Math. With a single-token key/value axis softmax is exactly 1, so each path
collapses to z_i = x_kv @ W_i^T + c_i with W_i = w_o_i @ wv_i. The LayerNorm
folds through the output projection, and since mu is LINEAR in x it folds
into the weights too:

    h = rs*(z @ Wg^T - mu*wbar + cc) + b2,   mu = (x_u.s1u + x_m.s1m + cs1)/2D
      = rs*(x_u @ A1'^T + x_m @ A2'^T + cc) + b2
    A1' = Wg1@W1 - wbar (x) s1u/2D   (and A2' likewise)

Only rs = 1/sqrt(var+eps) is nonlinear; var needs S2 = sum_j z_j^2 =
||R1 x_u||^2 + ||R2 x_m||^2 (+linear+const) with R from QR(W_i) — upper
triangular, so 38% of its k-blocks are skipped.

Precision plan (tolerance 2e-2; this lands ~5e-3):
  - y-path: 3-term fp8 split (Ah.xh + Al.xh + Ah.xl) sharing one PSUM scale
    S = alpha*beta; each matmul is DoubleRow fp8e4 (0.5 cyc/row, 256-deep).
  - variance path entirely single fp8 (needs only ~1% accuracy): u = R8.xh,
    squares on ScalarE into fp8, ones-matvec + s1-matvec as DoubleRow fp8.
  - rs broadcast to 128 partitions via two bf16 (hi+residual) rank-1
    matmuls, exact to ~16 mantissa bits.

Per-chunk PE cost: 192*256 (y) + 40*256 (u, triangular) + 16*256
(matvecs) + 2*512 (bf16 rs broadcast) ~ 63.7k cycles -> ~26.5us/chunk,
4 chunks/core. Stats for chunk c+1 are interleaved tile-by-tile with the
y-matmuls of chunk c so rs is always ready ahead of its consumers; DMAs
are packed/ordered so the first u-matmul starts ~4us in and the A
tensors stream behind the x/R fills. Cost-model timeline: ~123us/core
(baseline all-bf16 design: 278.7us). HW-verified absmax-rel ~6.1e-3.
"""

import sys

sys.path.insert(0, "/opt/trn_rl_repo")

import ml_dtypes
import numpy as np

import concourse.bass as bass
import concourse.mybir as mybir
import concourse.tile as tile
from concourse.bass import ts
from concourse.bass_utils import run_bass_kernel_spmd

N_CORES = 8
B = 16384
D = 1024
BC = B // N_CORES          # batch rows per core
NCHUNK = 512               # batch columns per chunk (one PSUM bank)
NCH = BC // NCHUNK         # chunks per core
MT = D // 128              # output m-tiles (8)
KX = D // 256              # DoubleRow k-steps over one input (4)
KS = 2 * D // 256          # DoubleRow k-steps over both (8) for matvecs
LN_EPS = 1e-5

F8 = mybir.dt.float8e4
F32 = mybir.dt.float32
F32R = mybir.dt.float32r
BF16 = mybir.dt.bfloat16
nf8 = ml_dtypes.float8_e4m3
DR = mybir.MatmulPerfMode.DoubleRow

ALU = mybir.AluOpType
AF = mybir.ActivationFunctionType


def split_multi_waits(nc):
    """This walrus build only honors one sync-wait per instruction. Move any
    extra waits onto same-engine NOPs inserted immediately before."""
    for f in nc.m.functions:
        for bb in f.blocks:
            new_insts = []
            changed = False
            for inst in bb.instructions:
                si = inst.sync_info
                waits = list(si.on_wait) if si and si.on_wait else []
                if len(waits) > 1:
                    changed = True
                    for w in waits[:-1]:
                        nop = mybir.InstNoOp(
                            name=nc.get_next_instruction_name(), ins=[], outs=[]
                        )
                        nop.engine = inst.engine
                        nop.sync_info = mybir.SyncInfo(on_wait=[w], on_update=[])
                        nc.register_instruction(nop)
                        new_insts.append(nop)
                    si.on_wait = waits[-1:]
                new_insts.append(inst)
            if changed:
                bb.instructions[:] = new_insts


def build_program(sc):
    """sc: dict of host-computed scale/bias floats baked into the program."""
    nc = bass.Bass("TRN2", target_bir_lowering=False, debug=False)

    def dram(name, shape, dt=F8):
        return nc.dram_tensor(name, shape, dt, kind="ExternalInput").ap()

    xhi = dram("xhi", [2 * D, BC]).rearrange(
        "(tk i p) n -> p tk i n", i=2, p=128)
    xlo = dram("xlo", [2 * D, BC]).rearrange(
        "(tk i p) n -> p tk i n", i=2, p=128)
    a1h = dram("a1h", [D, D]).rearrange("(k i p) m -> p k i m", i=2, p=128)
    a1l = dram("a1l", [D, D]).rearrange("(k i p) m -> p k i m", i=2, p=128)
    a2h = dram("a2h", [D, D]).rearrange("(k i p) m -> p k i m", i=2, p=128)
    a2l = dram("a2l", [D, D]).rearrange("(k i p) m -> p k i m", i=2, p=128)
    r1d = dram("r1", [D, D]).rearrange("(k i p) m -> p k i m", i=2, p=128)
    r2d = dram("r2", [D, D]).rearrange("(k i p) m -> p k i m", i=2, p=128)
    s1wd = dram("s1w", [2 * D, 16]).rearrange(
        "(tk i p) j -> p tk i j", i=2, p=128)
    cpd = dram("cpack", [128, 2 * MT], F32)
    out = nc.dram_tensor("outT", [D, BC], BF16, kind="ExternalOutput").ap()
    out3 = out.rearrange("(m p) n -> p m n", p=128)

    with tile.TileContext(nc) as tc:
        with (
            tc.tile_pool(name="wconst", bufs=1) as wconst,
            tc.tile_pool(name="xin", bufs=3) as xin,
            tc.tile_pool(name="usqp", bufs=2) as usqp,
            tc.tile_pool(name="srow", bufs=2) as srow,
            tc.tile_pool(name="rsbp", bufs=3) as rsbp,
            tc.tile_pool(name="t1p", bufs=3) as t1p,
            tc.tile_pool(name="outp", bufs=2) as outp,
            tc.tile_pool(name="yps", bufs=3, space="PSUM") as yps,
            tc.tile_pool(name="ups", bufs=3, space="PSUM") as ups,
            tc.tile_pool(name="sps", bufs=1, space="PSUM") as sps,
            tc.tile_pool(name="bps", bufs=1, space="PSUM") as bps,
        ):
            # ---- resident constants ----
            a_sb = {}
            for nm, src in (("a1h", a1h), ("a1l", a1l), ("a2h", a2h),
                            ("a2l", a2l)):
                a_sb[nm] = wconst.tile([128, KX, 2, D], F8, name=nm + "_sb")
            r1_sb = wconst.tile([128, KX, 2, D], F8)
            r2_sb = wconst.tile([128, KX, 2, D], F8)
            s1w_sb = wconst.tile([128, 2 * KX, 2, 16], F8)
            cp_sb = wconst.tile([128, 2 * MT], F32)
            onesb_sb = wconst.tile([1, 128], BF16)
            nc.vector.memset(onesb_sb[:], 1.0)
            ones8_sb = wconst.tile([128, 2, 16], F8)
            nc.vector.memset(ones8_sb[:], 1.0)
            seps_sb = wconst.tile([1, 1], F32)
            nc.vector.memset(seps_sb[:], float(sc["sd_bias"]))

            # weight DMAs in first-needed order: xuh + r1[k3] gate the
            # very first u-matmul (prologue emits u-tiles m-descending, so
            # tile m needs only k >= m//2). Triangular: k covers m <= 2k+1.
            def emit_rdma(r_sb, rd):
                for k in range(KX - 1, -1, -1):
                    mhi = min(D, (2 * k + 2) * 128)
                    nc.sync.dma_start(r_sb[:, k, :, :mhi], rd[:, k, :, :mhi])

            def emit_const_dma():
                nc.sync.dma_start(s1w_sb[:], s1wd[:])
                nc.sync.dma_start(cp_sb[:], cpd[:])
            adram = {"a1h": a1h, "a1l": a1l, "a2h": a2h, "a2l": a2l}

            def emit_adma(half):
                sl = slice(half * 512, half * 512 + 512)
                for nm in ("a1h", "a2h", "a1l", "a2l"):
                    nc.sync.dma_start(
                        a_sb[nm][:, :, :, sl], adram[nm][:, :, :, sl]
                    )

            x_sb = [None] * NCH      # per-chunk dict of x tiles
            o_sb = [None] * NCH
            usq_sb = [None] * NCH
            rsb = [None] * NCH       # rs broadcast SBUF tile
            s1p_t = [None] * NCH
            s2p_t = [None] * NCH
            rrow = [None] * NCH

            def emit_xdma(ci, parts=("hi", "lo")):
                n0 = ci * NCHUNK
                nsl = slice(n0, n0 + NCHUNK)
                srcs = {"hi": xhi, "lo": xlo}
                if x_sb[ci] is None:
                    x_sb[ci] = {}
                    usq_sb[ci] = usqp.tile(
                        [128, KS, 2, NCHUNK], F8, name="usq_t", tag="usq")
                    o_sb[ci] = outp.tile(
                        [128, MT, NCHUNK], BF16, name="o_t", tag="o")
                for nm in parts:
                    if nm in ("hi", "lo"):
                        t = xin.tile([128, 2 * KX, 2, NCHUNK], F8,
                                     name="x" + nm + "_t", tag="x" + nm)
                        nc.sync.dma_start(t[:], srcs[nm][:, :, :, nsl])
                        x_sb[ci][nm] = t
                    else:
                        # half-pack slices for the prologue: "hi_u"/"hi_m"
                        part, tk = nm.split("_")
                        tksl = slice(0, KX) if tk == "u" else slice(KX, 2 * KX)
                        if part + "_t" not in x_sb[ci]:
                            x_sb[ci][part] = xin.tile(
                                [128, 2 * KX, 2, NCHUNK], F8,
                                name="x" + part + "_t", tag="x" + part)
                            x_sb[ci][part + "_t"] = True
                        nc.sync.dma_start(
                            x_sb[ci][part][:, tksl, :, :],
                            srcs[part][:, tksl, :, nsl],
                        )

            def emit_u(ci, m, only_path=None):
                """u-tile m + fp8 squares (Act path 0, DVE path 1)."""
                xt = x_sb[ci]
                for path, r_sb in enumerate((r1_sb, r2_sb)):
                    if only_path is not None and path != only_path:
                        continue
                    xh = xt["hi"]
                    ko = path * KX
                    up = ups.tile([128, NCHUNK], F32, name="up", tag="up")
                    ks = range(m // 2, KX)
                    for k in ks:
                        nc.tensor.matmul(
                            up[:],
                            lhsT=r_sb[:, k, :, ts(m, 128)],
                            rhs=xh[:, ko + k, :, :],
                            start=(k == m // 2),
                            stop=(k == KX - 1),
                            perf_mode=DR,
                        )
                    dst = usq_sb[ci][:, path * KX + m // 2, m % 2, :]
                    nc.scalar.activation(
                        dst, up[:], AF.Square, bias=0.0, scale=sc["s_act"],
                    )

            def emit_s1(ci):
                xt = x_sb[ci]
                s1pp = sps.tile([16, NCHUNK], F32, name="s1pp", tag="sp")
                for k in range(2 * KX):
                    nc.tensor.matmul(
                        s1pp[:], lhsT=s1w_sb[:, k, :, :],
                        rhs=xt["hi"][:, k, :, :],
                        start=(k == 0), stop=(k == 2 * KX - 1), perf_mode=DR,
                    )
                s1p_t[ci] = s1pp
                # mu and mu^2 rows
                mu = srow.tile([1, NCHUNK], F32, name="mu", tag="mu")
                nc.scalar.activation(
                    mu[:], s1pp[0:1, :], AF.Copy,
                    bias=float(sc["mu_bias"]), scale=sc["mu_scale"],
                )
                musq = srow.tile([1, NCHUNK], F32, name="musq", tag="musq")
                nc.vector.tensor_mul(musq[:], mu[:], mu[:])
                return musq

            def emit_s2_rs(ci, musq):
                s2pp = sps.tile([16, NCHUNK], F32, name="s2pp", tag="sp")
                for k in range(KS):
                    nc.tensor.matmul(
                        s2pp[:], lhsT=ones8_sb[:],
                        rhs=usq_sb[ci][:, k, :, :],
                        start=(k == 0), stop=(k == KS - 1), perf_mode=DR,
                    )
                s2p_t[ci] = s2pp
                var = srow.tile([1, NCHUNK], F32, name="var", tag="var")
                nc.vector.scalar_tensor_tensor(
                    out=var[:], in0=s2pp[0:1, :], scalar=sc["var_scale"],
                    in1=musq[:], op0=ALU.mult, op1=ALU.subtract,
                )
                sd2 = srow.tile([1, NCHUNK], F32, name="sd2", tag="sd2")
                nc.scalar.activation(
                    sd2[:], var[:], AF.Sqrt,
                    bias=seps_sb[:], scale=sc["sd_scale"],
                )
                rs = srow.tile([1, NCHUNK], F32, name="rs", tag="rs")
                nc.vector.reciprocal(rs[:], sd2[:])
                rrow[ci] = rs

            def emit_bcast(ci):
                # rs broadcast via two bf16 rank-1 matmuls (hi + residual):
                # 2x512 PE cycles instead of a 2048-cycle fp32 matmul.
                rhi = srow.tile([1, NCHUNK], BF16, name="rhi", tag="rhi")
                nc.vector.tensor_copy(out=rhi[:], in_=rrow[ci][:])
                rlo = srow.tile([1, NCHUNK], BF16, name="rlo", tag="rlo")
                nc.vector.scalar_tensor_tensor(
                    out=rlo[:], in0=rhi[:], scalar=-1.0, in1=rrow[ci][:],
                    op0=ALU.mult, op1=ALU.add,
                )
                rsp = bps.tile([128, NCHUNK], F32, name="rsp", tag="rsp")
                nc.tensor.matmul(
                    rsp[:], lhsT=onesb_sb[:], rhs=rhi[:],
                    start=True, stop=False,
                )
                nc.tensor.matmul(
                    rsp[:], lhsT=onesb_sb[:], rhs=rlo[:],
                    start=False, stop=True,
                )
                rb = rsbp.tile([128, NCHUNK], F32, name="rb", tag="rb")
                nc.vector.tensor_copy(out=rb[:], in_=rsp[:])
                rsb[ci] = rb

            ypt = {}

            def emit_ymm(ci, m):
                xt = x_sb[ci]
                yp = yps.tile([128, NCHUNK], F32, name="yp", tag="yp")
                groups = [
                    (a_sb["a1h"], "hi", 0), (a_sb["a2h"], "hi", KX),
                    (a_sb["a1l"], "hi", 0), (a_sb["a2l"], "hi", KX),
                    (a_sb["a1h"], "lo", 0), (a_sb["a2h"], "lo", KX),
                ]
                ng = len(groups)
                for gi, (w_sb, part, ko) in enumerate(groups):
                    for k in range(KX):
                        nc.tensor.matmul(
                            yp[:],
                            lhsT=w_sb[:, k, :, ts(m, 128)],
                            rhs=xt[part][:, ko + k, :, :],
                            start=(gi == 0 and k == 0),
                            stop=(gi == ng - 1 and k == KX - 1),
                            perf_mode=DR,
                        )
                ypt[(ci, m)] = yp

            def emit_ytail(ci, m):
                n0 = ci * NCHUNK
                yp = ypt.pop((ci, m))
                t1 = t1p.tile([128, NCHUNK], F32, name="t1", tag="t1")
                last = ci == NCH - 1 and m == MT - 1
                # the very last tile drains in column halves so its
                # t1/gelu/DMA chains pipeline instead of serializing
                cols = ((0, NCHUNK // 2), (NCHUNK // 2, NCHUNK)) if last \
                    else ((0, NCHUNK),)
                for c0, c1 in cols:
                    nc.vector.scalar_tensor_tensor(
                        out=t1[:, c0:c1], in0=yp[:, c0:c1],
                        scalar=cp_sb[:, MT + m : MT + m + 1],
                        in1=rsb[ci][:, c0:c1], op0=ALU.add, op1=ALU.mult,
                    )
                    nc.scalar.activation(
                        o_sb[ci][:, m, c0:c1], t1[:, c0:c1], AF.Gelu,
                        bias=cp_sb[:, m : m + 1],
                    )
                    if last:
                        nc.sync.dma_start(
                            out3[:, m : m + 1, n0 + c0 : n0 + c1],
                            o_sb[ci][:, m : m + 1, c0:c1],
                        )
                if last:
                    return
                # out DMA in halves; last chunk per-tile so the final
                # drain overlaps the remaining gelus
                if ci == NCH - 1:
                    nc.sync.dma_start(
                        out3[:, m : m + 1, n0 : n0 + NCHUNK],
                        o_sb[ci][:, m : m + 1, :],
                    )
                elif m == MT // 2 - 1 or m == MT - 1:
                    m0 = 0 if m < MT // 2 else MT // 2
                    nc.sync.dma_start(
                        out3[:, m0 : m + 1, n0 : n0 + NCHUNK],
                        o_sb[ci][:, m0 : m + 1, :],
                    )

            def emit_y(ci, m):
                emit_ymm(ci, m)
                emit_ytail(ci, m)

            # ---- prologue: r1+xuh landed first, u(0) path-1 starts
            # ASAP; chunk-1 stats and chunk-0 y-tiles interleave so PE
            # stays busy while the A tensors stream in ----
            # xuh in k-halves so the first u-matmuls (m descending, k>=2)
            # start before the low-k half lands
            n0sl = slice(0, NCHUNK)
            x_sb[0] = {}
            usq_sb[0] = usqp.tile([128, KS, 2, NCHUNK], F8, name="usq_t",
                                  tag="usq")
            o_sb[0] = outp.tile([128, MT, NCHUNK], BF16, name="o_t", tag="o")
            x_sb[0]["hi"] = xin.tile([128, 2 * KX, 2, NCHUNK], F8,
                                     name="xhi_t", tag="xhi")
            nc.sync.dma_start(r1_sb[:, 3, :, 768:], r1d[:, 3, :, 768:])
            nc.sync.dma_start(x_sb[0]["hi"][:, 2:4, :, :],
                              xhi[:, 2:4, :, n0sl])
            nc.sync.dma_start(r1_sb[:, 3, :, :768], r1d[:, 3, :, :768])
            nc.sync.dma_start(x_sb[0]["hi"][:, 0:2, :, :],
                              xhi[:, 0:2, :, n0sl])
            for k in range(KX - 2, -1, -1):
                mhi = min(D, (2 * k + 2) * 128)
                nc.sync.dma_start(r1_sb[:, k, :, :mhi], r1d[:, k, :, :mhi])
            emit_const_dma()
            nc.sync.dma_start(x_sb[0]["hi"][:, KX:, :, :],
                              xhi[:, KX:, :, n0sl])
            emit_rdma(r2_sb, r2d)
            emit_adma(0)
            emit_xdma(0, ("lo",))
            emit_xdma(1, ("hi",))
            emit_adma(1)
            emit_xdma(1, ("lo",))
            for m in range(MT - 1, -1, -1):
                emit_u(0, m, only_path=0)
            musq = emit_s1(0)
            for m in range(MT - 1, -1, -1):
                emit_u(0, m, only_path=1)
            emit_ymm(0, 0)
            emit_s2_rs(0, musq)
            emit_ymm(0, 1)
            emit_bcast(0)
            emit_ytail(0, 0)
            emit_ytail(0, 1)
            musq = None
            for m in range(2, MT):
                if m < 6:
                    emit_u(1, 2 * (m - 2))
                    emit_u(1, 2 * (m - 2) + 1)
                if m == 5:
                    musq = emit_s1(1)
                emit_y(0, m)
                if m == 6:
                    emit_s2_rs(1, musq)
                if m == 7:
                    emit_bcast(1)
            pending0 = -1

            # ---- steady state: chunk ci's y-tiles interleaved with chunk
            # ci+1's stats; u-tiles spread across all slots so ScalarE
            # squares never back up, s2+rs at iteration end, broadcast in
            # the next iteration's first slot ----
            pending = pending0
            for ci in range(1, NCH):
                nxt = ci + 1
                if nxt < NCH:
                    emit_xdma(nxt)
                musq = None
                for m in range(MT):
                    if nxt < NCH:
                        emit_u(nxt, m, only_path=0)
                        emit_u(nxt, m, only_path=1)
                        if m == MT // 2 - 1:
                            musq = emit_s1(nxt)
                    emit_ymm(ci, m)
                    if m == 0 and pending == ci:
                        emit_bcast(ci)
                        pending = -1
                    emit_ytail(ci, m)
                if nxt < NCH:
                    emit_s2_rs(nxt, musq)
                    pending = nxt

    split_multi_waits(nc)
    return nc


def fold_weights(inputs):
    """Host-side algebra: collapse both attention paths + LayerNorm folds,
    QR factors for the variance, fp8 hi/lo quantization with shared
    power-of-2 scales."""
    f64 = np.float64
    f32 = np.float32
    w_qkv1 = np.asarray(inputs["w_qkv1"], f64)
    w_qkv2 = np.asarray(inputs["w_qkv2"], f64)
    b_qkv1 = np.asarray(inputs["b_qkv1"], f64)
    b_qkv2 = np.asarray(inputs["b_qkv2"], f64)
    w_o1 = np.asarray(inputs["w_o1"], f64)
    w_o2 = np.asarray(inputs["w_o2"], f64)
    b_o1 = np.asarray(inputs["b_o1"], f64)
    b_o2 = np.asarray(inputs["b_o2"], f64)
    w_proj = np.asarray(inputs["w_proj"], f64)
    b_proj = np.asarray(inputs["b_proj"], f64)
    g = np.asarray(inputs["ln_g"], f64)
    lnb = np.asarray(inputs["ln_b"], f64)

    wv1, bv1 = w_qkv1[2 * D :], b_qkv1[2 * D :]
    wv2, bv2 = w_qkv2[2 * D :], b_qkv2[2 * D :]
    W1 = w_o1 @ wv1
    c1 = w_o1 @ bv1 + b_o1
    W2 = w_o2 @ wv2
    c2 = w_o2 @ bv2 + b_o2
    Wg = w_proj * g[None, :]
    A1 = Wg[:, :D] @ W1
    A2 = Wg[:, D:] @ W2
    wbar = Wg.sum(axis=1)
    s1u = W1.sum(axis=0)
    s1m = W2.sum(axis=0)
    cs1 = c1.sum() + c2.sum()
    # fold the mu*wbar rank-1 correction into the weights
    A1p = A1 - np.outer(wbar, s1u) / (2 * D)
    A2p = A2 - np.outer(wbar, s1m) / (2 * D)
    # per-feature constant added before the rs multiply:
    #   cy = Wg1 c1 + Wg2 c2 - cs1*wbar/2D
    cy = Wg[:, :D] @ c1 + Wg[:, D:] @ c2 - cs1 * wbar / (2 * D)
    b2c = w_proj @ lnb + b_proj
    # variance pieces: S2 = ||R1 x_u||^2 + ||R2 x_m||^2 + s2lin.x + cs2
    R1 = np.linalg.qr(W1, mode="r")
    R2 = np.linalg.qr(W2, mode="r")
    s2lu = 2.0 * (W1.T @ c1)
    s2lm = 2.0 * (W2.T @ c2)
    cs2 = float(c1 @ c1 + c2 @ c2)

    def p2(m, target=200.0):
        return float(2.0 ** np.floor(np.log2(target / max(m, 1e-30))))

    beta = 16.0
    alpha = p2(max(np.abs(A1p).max(), np.abs(A2p).max()))
    lam = p2(max(np.abs(R1).max(), np.abs(R2).max()))
    gam = p2(max(np.abs(s1u).max(), np.abs(s1m).max()))
    S = alpha * beta

    assert np.abs(s2lu).max() == 0.0 and np.abs(s2lm).max() == 0.0 and \
        cs2 == 0.0, "nonzero v-biases: s2lin path not emitted in program"

    def hilo(a, s):
        a = np.ascontiguousarray(a.T)  # -> [contraction, out] lhsT layout
        hi = np.asarray(s * a, f32).astype(nf8)
        lo = (np.asarray(s * a, f32) - hi.astype(f32)).astype(nf8)
        return hi, lo

    a1h, a1l = hilo(A1p, alpha)
    a2h, a2l = hilo(A2p, alpha)
    r1q = np.ascontiguousarray(R1.T * lam).astype(f32).astype(nf8)
    r2q = np.ascontiguousarray(R2.T * lam).astype(f32).astype(nf8)
    s1w = np.zeros((2 * D, 16), f32)
    s1w[:D, 0] = gam * s1u
    s1w[D:, 0] = gam * s1m
    cpack = np.zeros((128, 2 * MT), f32)
    cpack[:, :MT] = b2c.reshape(MT, 128).T
    cpack[:, MT:] = (S * cy).reshape(MT, 128).T

    sc = {
        "s_act": 2.0 / (lam * beta),
        "mu_scale": 1.0 / (gam * beta * 2 * D),
        "mu_bias": cs1 / (2 * D),
        "var_scale": 1.0 / (4 * 2 * D),
        "sd_scale": S * S,
        "sd_bias": S * S * LN_EPS,
    }
    shared = {
        "a1h": a1h, "a1l": a1l, "a2h": a2h, "a2l": a2l,
        "r1": r1q, "r2": r2q,
        "s1w": s1w.astype(nf8),
        "cpack": np.ascontiguousarray(cpack),
    }
    return shared, sc, beta


_CACHED = {}


def _get_program(sc):
    key = tuple(sorted(sc.items()))
    if key not in _CACHED:
        _CACHED[key] = build_program(sc)
    return _CACHED[key]


def run(inputs, trace=False):
    """Quantize + shard inputs, run on 8 cores, gather the full output."""
    f32 = np.float32
    shared, sc, beta = fold_weights(inputs)
    x_u = np.asarray(inputs["x_u"], f32)
    x_m = np.asarray(inputs["x_m"], f32)

    xT = np.concatenate([x_u.T, x_m.T], axis=0)  # [2D, B]
    xhi = np.asarray(beta * xT, f32).astype(nf8)
    xlo = (np.asarray(beta * xT, f32) - xhi.astype(f32)).astype(nf8)

    in_maps = []
    for c in range(N_CORES):
        sl = slice(c * BC, (c + 1) * BC)
        m = dict(shared)
        m["xhi"] = np.ascontiguousarray(xhi[:, sl])
        m["xlo"] = np.ascontiguousarray(xlo[:, sl])
        in_maps.append(m)

    nc = _get_program(sc)
    res = run_bass_kernel_spmd(nc, in_maps, list(range(N_CORES)), trace=trace)
    out = np.empty((B, D), f32)
    for c in range(N_CORES):
        out[c * BC : (c + 1) * BC, :] = res.results[c]["outT"].astype(f32).T
    return out, res


def kernel(**inputs) -> np.ndarray:
    out, _ = run(inputs, trace=False)
    return out
